# revision 8
# baseline (speedup 1.0000x reference)
"""Batched same-batch KNN (top-3) + fused MLP for Trainium2, 8 NeuronCores.

Strategy
--------
Host side (numpy, exact):
  * Stable-group rows of a and b by batch id. Batch g -> core g (B == 8 ==
    n_cores). Within a batch the original relative order is preserved, so
    the device's tie handling matches jax.lax.top_k (first occurrence).
  * Packed-value distance trick: the K=6 matmul computes
        val[i,j] = -2048*d'[i,j] - j
    with ua = [-2048|xa|^2, 2048, 4096*xa, -1], vb = [1, -|xb|^2, xb, j].
    d' is the integer squared voxel distance (<= 3*127^2). For d' < 8192
    the packed value is an exact fp32 integer (< 2^24), so a single DVE
    max8 yields both the top-3 distances AND their column indices
    (j = low 11 bits) -- no FIND_INDEX8 pass. For d' >= 8192 rounding can
    scramble j/tie order, but those pairs have dw == relu(.5 - d/16384)
    == 0 exactly, so any selection among them is output-equivalent.
Device side (per core, SPMD):
  * R = relu(feats_bg @ W1) (b1 == 0 asserted; numpy fallback otherwise)
    written to a DRAM table.
  * Per 128-row a-tile: distance matmul in 2 PSUM halves, DVE max8 reads
    PSUM directly (no copy), merge max8, pool bit-ops decode (j, d),
    dw = relu(.5 - d/16384), ONE batched indirect-DMA gather of all 3 R
    rows (384 descriptors, one SWDGE fixed overhead), weighted sum with
    dw^2 spread over ACT/Pool/DVE, PE transpose, grouped @W2 (+3*b2).
Outputs are scattered back to original row order on host; the feats_a
passthrough half of the concat is host-side assembly.
"""

import os
import numpy as np

import concourse.bass as bass
import concourse.mybir as mybir
import concourse.tile as tile
from concourse import bacc
from concourse.bass import IndirectOffsetOnAxis
from concourse.bass_utils import run_bass_kernel_spmd
from concourse.masks import make_identity

P = 128
NPAD = 1664  # 13 * 128; covers per-batch row counts for Na=Nb=12288, B=8
NT = NPAD // P
HALF = 832  # distance PSUM half-width (2 banks)
DF = 256
TOPK = 3
FULL_SCALE = 128
RCLIP = 0.5
INV_SCALE2 = 1.0 / (FULL_SCALE * FULL_SCALE)
BIG = 1.0e9
N_CORES = 8
PACK = 2048.0  # index-packing scale; NPAD <= 2047
GPD = 272  # padded per-k stride in the gather dest tile (elems)

_PROGRAM_CACHE = {}


def _build_program():
    """Build the SPMD Bass program (identical on all 8 cores)."""
    nc = bacc.Bacc("TRN2", target_bir_lowering=False, debug=False)
    f32 = mybir.dt.float32
    u32 = mybir.dt.uint32

    uaT = nc.dram_tensor("uaT", [6, NPAD], f32, kind="ExternalInput").ap()
    vbT = nc.dram_tensor("vbT", [6, NPAD], f32, kind="ExternalInput").ap()
    fbT = nc.dram_tensor("fbT", [DF, NPAD], f32, kind="ExternalInput").ap()
    w1 = nc.dram_tensor("w1", [DF, DF], f32, kind="ExternalInput").ap()
    w2 = nc.dram_tensor("w2", [DF, DF], f32, kind="ExternalInput").ap()
    b2c3 = nc.dram_tensor("b2c3", [P, 2], f32, kind="ExternalInput").ap()
    fusedT = nc.dram_tensor("fusedT", [DF, NPAD], f32, kind="ExternalOutput").ap()

    with tile.TileContext(nc) as tc:
        with (
            tc.tile_pool(name="const", bufs=1) as cpool,
            tc.tile_pool(name="dram", bufs=1, space="DRAM") as dpool_dram,
        ):
            # --- input loads, spread across the two HWDGE queues (SP + ACT)
            # and ordered so phase R (fbT+w1) and the distance matmuls
            # (uaT/vbT) can start as early as possible.
            w1k0 = cpool.tile([P, DF], f32)
            w1k1 = cpool.tile([P, DF], f32)
            nc.sync.dma_start(w1k0[:], w1[0:P, :])
            nc.sync.dma_start(w1k1[:], w1[P : 2 * P, :])
            fbT0 = cpool.tile([P, NPAD], f32)
            fbT1 = cpool.tile([P, NPAD], f32)
            nc.scalar.dma_start(fbT0[:, 0:HALF], fbT[0:P, 0:HALF])
            nc.scalar.dma_start(fbT1[:, 0:HALF], fbT[P : 2 * P, 0:HALF])
            # coord operands replicated at partition offsets 0/32/64/96 so the
            # K=6 distance matmuls can run in 4 concurrent PE row-groups
            uaT4 = cpool.tile([102, NPAD], f32)
            vbT4 = cpool.tile([102, NPAD], f32)
            for gofs in (0, 32, 64, 96):
                nc.sync.dma_start(uaT4[gofs : gofs + 6, :], uaT[:])
                nc.scalar.dma_start(vbT4[gofs : gofs + 6, :], vbT[:])
            nc.scalar.dma_start(fbT0[:, HALF:NPAD], fbT[0:P, HALF:NPAD])
            nc.scalar.dma_start(fbT1[:, HALF:NPAD], fbT[P : 2 * P, HALF:NPAD])
            w2k0 = cpool.tile([P, DF], f32)
            w2k1 = cpool.tile([P, DF], f32)
            nc.sync.dma_start(w2k0[:], w2[0:P, :])
            nc.sync.dma_start(w2k1[:], w2[P : 2 * P, :])
            b2s = cpool.tile([P, 2], f32)
            nc.sync.dma_start(b2s[:], b2c3[:])
            ident = cpool.tile([P, P], f32)
            make_identity(nc, ident[:])
            zcol = cpool.tile([P, 1], f32)
            nc.vector.memset(zcol[:], 0.0)
            halfcol = cpool.tile([P, 1], f32)
            nc.vector.memset(halfcol[:], RCLIP)

            rtab = dpool_dram.tile([NPAD, DF], f32)

            # ---- Phase R: R = relu(feats_bg @ W1), row-major in DRAM
            with (
                tc.tile_pool(name="psR", bufs=2, space="PSUM") as psR_pool,
                tc.tile_pool(name="rsb", bufs=3) as r_pool,
            ):
                for t in range(NT):
                    sl = bass.ts(t, P)
                    psR = psR_pool.tile([P, DF], f32)
                    nc.tensor.matmul(
                        psR[:], lhsT=fbT0[:, sl], rhs=w1k0[:], start=True, stop=False
                    )
                    nc.tensor.matmul(
                        psR[:], lhsT=fbT1[:, sl], rhs=w1k1[:], start=False, stop=True
                    )
                    rt = r_pool.tile([P, DF], f32)
                    nc.scalar.activation(
                        rt[:], psR[:], mybir.ActivationFunctionType.Relu, bias=zcol[:]
                    )
                    nc.sync.dma_start(rtab[sl, :], rt[:])

            # ---- Phase D: packed distances, top-3, batched gather, W2.
            GRP = 4
            with (
                tc.tile_pool(name="dps", bufs=2, space="PSUM") as d_pool,
                tc.tile_pool(name="tps", bufs=2, space="PSUM") as t_pool,
                tc.tile_pool(name="fps", bufs=1, space="PSUM") as f_pool,
                tc.tile_pool(name="small", bufs=8) as s_pool,
                tc.tile_pool(name="gat", bufs=6) as g_pool,
                tc.tile_pool(name="accp", bufs=2) as a_pool,
                tc.tile_pool(name="outp", bufs=2) as o_pool,
            ):
                state = {}

                def topk_and_gather(t):
                    sl = bass.ts(t, P)
                    vals16 = s_pool.tile([P, 16], f32, tag="v16")
                    for h in (0, 1):
                        dps = d_pool.tile([P, HALF], f32)
                        for ci, (c0, c1) in enumerate(((0, 512), (512, HALF))):
                            gofs = 32 * ((2 * h + ci) % 4)
                            nc.tensor.matmul(
                                dps[:, c0:c1],
                                lhsT=uaT4[gofs : gofs + 6, sl],
                                rhs=vbT4[gofs : gofs + 6, h * HALF + c0 : h * HALF + c1],
                                start=True,
                                stop=True,
                                tile_position=(gofs, 0),
                            )
                        # DVE max8 straight from PSUM; packed top-8 values
                        nc.vector.max(out=vals16[:, 8 * h : 8 * h + 8], in_=dps[:])
                    vals8 = s_pool.tile([P, 8], f32, tag="v8")
                    nc.vector.max(out=vals8[:], in_=vals16[:])
                    # decode: V = -val = 2048*d + j  (exact int where it matters)
                    vu = s_pool.tile([P, TOPK], u32, tag="vu")
                    nc.gpsimd.tensor_scalar(
                        out=vu[:], in0=vals8[:, 0:TOPK], scalar1=-1.0, scalar2=None,
                        op0=mybir.AluOpType.mult,
                    )
                    j3 = s_pool.tile([P, TOPK], u32, tag="j3")
                    nc.vector.tensor_scalar(
                        out=j3[:], in0=vu[:], scalar1=2047, scalar2=None,
                        op0=mybir.AluOpType.bitwise_and,
                    )
                    df3u = s_pool.tile([P, TOPK], u32, tag="dfu")
                    nc.vector.tensor_scalar(
                        out=df3u[:], in0=vu[:], scalar1=11, scalar2=None,
                        op0=mybir.AluOpType.logical_shift_right,
                    )
                    df3 = s_pool.tile([P, TOPK], f32, tag="df")
                    nc.gpsimd.tensor_copy(out=df3[:], in_=df3u[:])
                    dw = s_pool.tile([P, TOPK], f32, tag="dw")
                    nc.scalar.activation(
                        dw[:],
                        df3[:],
                        mybir.ActivationFunctionType.Relu,
                        bias=halfcol[:],
                        scale=-INV_SCALE2,
                    )
                    dw2 = s_pool.tile([P, TOPK], f32, tag="dw2")
                    nc.scalar.activation(
                        dw2[:], dw[:], mybir.ActivationFunctionType.Square
                    )
                    # 3 indirect gathers (the SWDGE lowering supports exactly
                    # one offset per 128-partition descriptor set)
                    g = g_pool.tile([P, TOPK, DF], f32, tag="g")
                    for k in range(TOPK):
                        nc.gpsimd.indirect_dma_start(
                            out=g[:, k, :],
                            out_offset=None,
                            in_=rtab[:],
                            in_offset=IndirectOffsetOnAxis(ap=j3[:, k : k + 1], axis=0),
                        )
                    state[t] = (dw2, g)

                def mlp_group(g0, g1):
                    ntile = g1 - g0
                    accT0 = a_pool.tile([P, GRP * P], f32, tag="accT0")
                    accT1 = a_pool.tile([P, GRP * P], f32, tag="accT1")
                    for i, s in enumerate(range(g0, g1)):
                        dw2, g = state.pop(s)
                        acc = a_pool.tile([P, DF], f32, tag="acc")
                        tmp1 = a_pool.tile([P, DF], f32, tag="tmp1")
                        nc.scalar.mul(acc[:], g[:, 0, 0:DF], dw2[:, 0:1])
                        nc.scalar.mul(tmp1[:], g[:, 1, 0:DF], dw2[:, 1:2])
                        nc.gpsimd.tensor_add(acc[:], acc[:], tmp1[:])
                        nc.vector.scalar_tensor_tensor(
                            out=acc[:],
                            in0=g[:, 2, 0:DF],
                            scalar=dw2[:, 2:3],
                            in1=acc[:],
                            op0=mybir.AluOpType.mult,
                            op1=mybir.AluOpType.add,
                        )
                        for m, accTm in enumerate((accT0, accT1)):
                            pt = t_pool.tile([P, P], f32)
                            nc.tensor.transpose(
                                out=pt[:],
                                in_=acc[:, m * P : (m + 1) * P],
                                identity=ident[:],
                            )
                            nc.scalar.copy(accTm[:, bass.ts(i, P)], pt[:])
                    csl = slice(g0 * P, g1 * P)
                    for m in range(2):
                        msl = bass.ts(m, P)
                        pf = f_pool.tile([P, GRP * P], f32)
                        nc.tensor.matmul(
                            pf[:, : ntile * P],
                            lhsT=w2k0[:, msl],
                            rhs=accT0[:, : ntile * P],
                            start=True,
                            stop=False,
                        )
                        nc.tensor.matmul(
                            pf[:, : ntile * P],
                            lhsT=w2k1[:, msl],
                            rhs=accT1[:, : ntile * P],
                            start=False,
                            stop=True,
                        )
                        oT = o_pool.tile([P, GRP * P], f32)
                        nc.scalar.activation(
                            oT[:, : ntile * P],
                            pf[:, : ntile * P],
                            mybir.ActivationFunctionType.Identity,
                            bias=b2s[:, m : m + 1],
                        )
                        nc.sync.dma_start(fusedT[msl, csl], oT[:, : ntile * P])

                SKEW = 2
                done = 0
                for t0 in range(0, NT + SKEW + 1, 2):
                    for t in (t0, t0 + 1):
                        if t < NT:
                            topk_and_gather(t)
                    t = min(t0 + 1, NT + SKEW)
                    # flush any complete group whose gathers are >= SKEW old
                    while done < NT and ((min(done + GRP, NT) - 1) + SKEW <= t):
                        g1 = min(done + GRP, NT)
                        mlp_group(done, g1)
                        done = g1
    nc.compile()
    return nc


def get_program():
    if "nc" not in _PROGRAM_CACHE:
        _PROGRAM_CACHE["nc"] = _build_program()
    return _PROGRAM_CACHE["nc"]


def _host_prep(batch_a, coords_a, batch_b, coords_b, feats_b, W1, b1, W2, b2):
    """Group by batch, build per-core input arrays. Returns (in_maps, meta)."""
    pa = np.argsort(batch_a, kind="stable")
    pb = np.argsort(batch_b, kind="stable")
    ca = np.bincount(batch_a, minlength=N_CORES)
    cb = np.bincount(batch_b, minlength=N_CORES)
    oa = np.concatenate([[0], np.cumsum(ca)])
    ob = np.concatenate([[0], np.cumsum(cb)])

    b2c3 = np.ascontiguousarray((3.0 * b2).astype(np.float32).reshape(2, P).T)
    w1c = np.ascontiguousarray(W1.astype(np.float32))
    w2c = np.ascontiguousarray(W2.astype(np.float32))

    in_maps = []
    meta = []
    for g in range(N_CORES):
        a_idx = pa[oa[g] : oa[g + 1]]
        b_idx = pb[ob[g] : ob[g + 1]]
        na, nb = len(a_idx), len(b_idx)
        if na > NPAD or nb > NPAD or (0 < nb < TOPK):
            return None, None  # shapes outside the compiled envelope -> fallback
        xa = (coords_a[a_idx] // 16).astype(np.float32)
        xb = (coords_b[b_idx] // 16).astype(np.float32)

        uaT = np.zeros((6, NPAD), dtype=np.float32)
        uaT[1, :] = PACK
        uaT[5, :] = -1.0
        if na > 0:
            uaT[0, :na] = -PACK * np.square(xa).sum(1)
            uaT[2:5, :na] = (2.0 * PACK * xa).T
            # pad a-cols: copy of column 0 (harmless rows, outputs dropped)
            if na < NPAD:
                uaT[:, na:] = uaT[:, :1]

        vbT = np.zeros((6, NPAD), dtype=np.float32)
        vbT[0, :] = 1.0
        vbT[1, :] = -BIG  # pad cols: huge distance, never selected
        vbT[5, :] = np.arange(NPAD, dtype=np.float32)
        if nb > 0:
            vbT[1, :nb] = -np.square(xb).sum(1)
            vbT[2:5, :nb] = xb.T

        fbT = np.zeros((DF, NPAD), dtype=np.float32)
        if nb > 0:
            fbT[:, :nb] = feats_b[b_idx].T

        in_maps.append(
            {
                "uaT": uaT,
                "vbT": vbT,
                "fbT": fbT,
                "w1": w1c,
                "w2": w2c,
                "b2c3": b2c3,
            }
        )
        meta.append((a_idx, na, nb))
    return in_maps, meta


def _reference_numpy(batch_a, coords_a, feats_a, batch_b, coords_b, feats_b,
                     W1, b1, W2, b2):
    """Exact numpy fallback (mirrors reference.py) for out-of-envelope data."""
    xa = (coords_a // 16).astype(np.float32)
    xb = (coords_b // 16).astype(np.float32)
    d = (
        np.square(xa).sum(1)[:, None]
        + np.square(xb).sum(1)[None, :]
        - 2.0 * (xa @ xb.T)
    )
    d = np.clip(d, 0.0, None) / (FULL_SCALE**2)
    same = batch_a[:, None] == batch_b[None, :]
    d = np.where(same, d, np.inf)
    idx = np.argsort(d, axis=1, kind="stable")[:, :TOPK]
    dv = np.take_along_axis(d, idx, axis=1)
    dwt = RCLIP - np.clip(dv, 0.0, RCLIP)
    b_f = feats_b[idx] * dwt[..., None]
    h = np.maximum(b_f @ W1 + b1, 0.0) * dwt[..., None]
    fused = (h @ W2 + b2).sum(axis=1)
    return np.concatenate([feats_a, fused], axis=1).astype(np.float32)


def _ensure_ntff_hook():
    """Install the axon NTFF profile hook (missing antenv.axon_hooks shim)."""
    import sys
    import types

    if "antenv.axon_hooks" in sys.modules:
        return
    try:
        from trn_agent_boot.trn_boot import _ntff_profile_via_ctypes

        hook = _ntff_profile_via_ctypes("/opt/axon/libaxon_pjrt.so")
    except Exception:
        hook = None
    mod = types.ModuleType("antenv.axon_hooks")
    _state = {"hook": hook}
    mod.get_axon_ntff_profile_hook = lambda: _state["hook"]

    def _set(h):
        _state["hook"] = h

    mod.set_axon_ntff_profile_hook = _set
    sys.modules["antenv.axon_hooks"] = mod


def kernel(batch_a, coords_a, feats_a, batch_b, coords_b, feats_b, W1, b1, W2, b2):
    batch_a = np.asarray(batch_a)
    coords_a = np.asarray(coords_a)
    feats_a = np.asarray(feats_a, dtype=np.float32)
    batch_b = np.asarray(batch_b)
    coords_b = np.asarray(coords_b)
    feats_b = np.asarray(feats_b, dtype=np.float32)
    W1 = np.asarray(W1, dtype=np.float32)
    b1 = np.asarray(b1, dtype=np.float32)
    W2 = np.asarray(W2, dtype=np.float32)
    b2 = np.asarray(b2, dtype=np.float32)

    if np.any(b1 != 0.0):
        # device pipeline folds dw through relu; exact only for b1 == 0
        return _reference_numpy(
            batch_a, coords_a, feats_a, batch_b, coords_b, feats_b, W1, b1, W2, b2
        )

    in_maps, meta = _host_prep(
        batch_a, coords_a, batch_b, coords_b, feats_b, W1, b1, W2, b2
    )
    if in_maps is None:
        return _reference_numpy(
            batch_a, coords_a, feats_a, batch_b, coords_b, feats_b, W1, b1, W2, b2
        )

    nc = get_program()
    trace = bool(int(os.environ.get("KERNEL_TRACE", "0")))
    if trace:
        _ensure_ntff_hook()
    res = run_bass_kernel_spmd(
        nc, in_maps, core_ids=list(range(N_CORES)), trace=trace
    )
    kernel.last_results = res

    fused = np.zeros((len(batch_a), DF), dtype=np.float32)
    for g in range(N_CORES):
        a_idx, na, nb = meta[g]
        if na == 0:
            continue
        out_g = res.results[g]["fusedT"]  # [DF, NPAD]
        if nb == 0:
            # reference: dw=0 rows -> h=0 -> fused = 3*b2
            fused[a_idx] = 3.0 * b2
        else:
            fused[a_idx] = out_g[:, :na].T
    return np.concatenate([feats_a, fused], axis=1)


# revision 9
# speedup vs baseline: 1.2567x; 1.2567x over previous
"""Batched same-batch KNN (top-3) + fused MLP for Trainium2, 8 NeuronCores.

Strategy
--------
Host side (numpy, exact):
  * Stable-group rows of a and b by batch id. Batch g -> core g (B == 8 ==
    n_cores). Within a batch the original relative order is preserved, so
    the device's tie handling matches jax.lax.top_k (first occurrence).
  * Packed-value distance trick: the K=6 matmul computes
        val[i,j] = -2048*d'[i,j] - j
    with ua = [-2048|xa|^2, 2048, 4096*xa, -1], vb = [1, -|xb|^2, xb, j].
    d' is the integer squared voxel distance (<= 3*127^2). For d' < 8192
    the packed value is an exact fp32 integer (< 2^24), so a single DVE
    max8 yields both the top-3 distances AND their column indices
    (j = low 11 bits) -- no FIND_INDEX8 pass. For d' >= 8192 rounding can
    scramble j/tie order, but those pairs have dw == relu(.5 - d/16384)
    == 0 exactly, so any selection among them is output-equivalent.
Device side (per core, SPMD):
  * R = relu(feats_bg @ W1) (b1 == 0 asserted; numpy fallback otherwise)
    written to a DRAM table.
  * Per 128-row a-tile: distance matmul in 2 PSUM halves, DVE max8 reads
    PSUM directly (no copy), merge max8, pool bit-ops decode (j, d),
    dw = relu(.5 - d/16384), ONE batched indirect-DMA gather of all 3 R
    rows (384 descriptors, one SWDGE fixed overhead), weighted sum with
    dw^2 spread over ACT/Pool/DVE, PE transpose, grouped @W2 (+3*b2).
Outputs are scattered back to original row order on host; the feats_a
passthrough half of the concat is host-side assembly.
"""

import os
import numpy as np

import concourse.bass as bass
import concourse.mybir as mybir
import concourse.tile as tile
from concourse import bacc
from concourse.bass import IndirectOffsetOnAxis
from concourse.bass_utils import run_bass_kernel_spmd
from concourse.masks import make_identity

P = 128
NPAD = 1664  # 13 * 128; covers per-batch row counts for Na=Nb=12288, B=8
NT = NPAD // P
HALF = 832  # distance PSUM half-width (2 banks)
DF = 256
TOPK = 3
FULL_SCALE = 128
RCLIP = 0.5
INV_SCALE2 = 1.0 / (FULL_SCALE * FULL_SCALE)
BIG = 1.0e9
N_CORES = 8
PACK = 2048.0  # index-packing scale; NPAD <= 2047
DPW = 1024  # dist PSUM tile width (2 full banks, bank-aligned bufs)

_PROGRAM_CACHE = {}


def _build_program():
    """Build the SPMD Bass program (identical on all 8 cores)."""
    nc = bacc.Bacc("TRN2", target_bir_lowering=False, debug=False)
    f32 = mybir.dt.float32
    u32 = mybir.dt.uint32
    bf16 = mybir.dt.bfloat16

    # uaT/vbT come pre-replicated x4 (partition offsets 0/32/64/96)
    uaT = nc.dram_tensor("uaT", [102, NPAD], f32, kind="ExternalInput").ap()
    vbT = nc.dram_tensor("vbT", [102, NPAD], f32, kind="ExternalInput").ap()
    fbT = nc.dram_tensor("fbT", [DF, NPAD], bf16, kind="ExternalInput").ap()
    w1 = nc.dram_tensor("w1", [DF, DF], bf16, kind="ExternalInput").ap()
    w2 = nc.dram_tensor("w2", [DF, DF], bf16, kind="ExternalInput").ap()
    b2c3 = nc.dram_tensor("b2c3", [P, 2], f32, kind="ExternalInput").ap()
    fusedT = nc.dram_tensor("fusedT", [DF, NPAD], f32, kind="ExternalOutput").ap()

    with tile.TileContext(nc) as tc:
        with (
            tc.tile_pool(name="const", bufs=1) as cpool,
            tc.tile_pool(name="dram", bufs=1, space="DRAM") as dpool_dram,
        ):
            # --- input loads, spread across the two HWDGE queues (SP + ACT),
            # ordered so phase R (w1+fbT) and the distance matmuls (uaT/vbT)
            # start as early as possible.
            w1k0 = cpool.tile([P, DF], bf16)
            w1k1 = cpool.tile([P, DF], bf16)
            fbT0 = cpool.tile([P, NPAD], bf16)
            fbT1 = cpool.tile([P, NPAD], bf16)
            nc.sync.dma_start(w1k0[:], w1[0:P, :])
            nc.sync.dma_start(w1k1[:], w1[P : 2 * P, :])
            nc.sync.dma_start(fbT0[:, 0:HALF], fbT[0:P, 0:HALF])
            nc.sync.dma_start(fbT1[:, 0:HALF], fbT[P : 2 * P, 0:HALF])
            uaT4 = cpool.tile([102, NPAD], f32)
            vbT4 = cpool.tile([102, NPAD], f32)
            nc.scalar.dma_start(uaT4[:], uaT[:])
            nc.scalar.dma_start(vbT4[:], vbT[:])
            nc.scalar.dma_start(fbT0[:, HALF:NPAD], fbT[0:P, HALF:NPAD])
            nc.scalar.dma_start(fbT1[:, HALF:NPAD], fbT[P : 2 * P, HALF:NPAD])
            w2k0 = cpool.tile([P, DF], bf16)
            w2k1 = cpool.tile([P, DF], bf16)
            nc.sync.dma_start(w2k0[:], w2[0:P, :])
            nc.sync.dma_start(w2k1[:], w2[P : 2 * P, :])
            b2s = cpool.tile([P, 2], f32)
            nc.sync.dma_start(b2s[:], b2c3[:])
            identb = cpool.tile([P, P], bf16)
            make_identity(nc, identb[:])
            zcol = cpool.tile([P, 1], f32)
            nc.vector.memset(zcol[:], 0.0)
            halfcol = cpool.tile([P, 1], f32)
            nc.vector.memset(halfcol[:], RCLIP)

            rtab = dpool_dram.tile([NPAD, DF], bf16)

            # ---- Phase R: R = relu(feats_bg @ W1), bf16 row-major in DRAM
            with (
                tc.tile_pool(name="psR", bufs=2, space="PSUM") as psR_pool,
                tc.tile_pool(name="rsb", bufs=3) as r_pool,
            ):
                for t in range(NT):
                    sl = bass.ts(t, P)
                    psR = psR_pool.tile([P, DF], f32)
                    nc.tensor.matmul(
                        psR[:], lhsT=fbT0[:, sl], rhs=w1k0[:], start=True, stop=False
                    )
                    nc.tensor.matmul(
                        psR[:], lhsT=fbT1[:, sl], rhs=w1k1[:], start=False, stop=True
                    )
                    rt = r_pool.tile([P, DF], bf16)
                    nc.scalar.activation(
                        rt[:], psR[:], mybir.ActivationFunctionType.Relu, bias=zcol[:]
                    )
                    nc.sync.dma_start(rtab[sl, :], rt[:])

            # ---- Phase D: packed distances, top-3, gathers, fused W2.
            GRP = 4
            with (
                tc.tile_pool(name="dps", bufs=2, space="PSUM") as d_pool,
                tc.tile_pool(name="tps", bufs=2, space="PSUM") as t_pool,
                tc.tile_pool(name="fps", bufs=1, space="PSUM") as f_pool,
                tc.tile_pool(name="small", bufs=8) as s_pool,
                tc.tile_pool(name="gat", bufs=6) as g_pool,
                tc.tile_pool(name="accp", bufs=2) as a_pool,
                tc.tile_pool(name="outp", bufs=2) as o_pool,
            ):
                state = {}

                def topk_and_gather(t):
                    sl = bass.ts(t, P)
                    vals16 = s_pool.tile([P, 16], f32, tag="v16")
                    for h in (0, 1):
                        dps = d_pool.tile([P, DPW], f32)
                        for ci, (c0, c1) in enumerate(((0, 512), (512, HALF))):
                            gofs = 32 * ((2 * h + ci) % 4)
                            nc.tensor.matmul(
                                dps[:, c0:c1],
                                lhsT=uaT4[gofs : gofs + 6, sl],
                                rhs=vbT4[gofs : gofs + 6, h * HALF + c0 : h * HALF + c1],
                                start=True,
                                stop=True,
                                tile_position=(gofs, 0),
                            )
                        # DVE max8 straight from PSUM; packed top-8 values
                        nc.vector.max(out=vals16[:, 8 * h : 8 * h + 8], in_=dps[:, 0:HALF])
                    vals8 = s_pool.tile([P, 8], f32, tag="v8")
                    nc.vector.max(out=vals8[:], in_=vals16[:])
                    # decode: V = -val = 2048*d + j  (exact int where it matters)
                    vu = s_pool.tile([P, TOPK], u32, tag="vu")
                    nc.vector.tensor_scalar(
                        out=vu[:], in0=vals8[:, 0:TOPK], scalar1=-1.0, scalar2=None,
                        op0=mybir.AluOpType.mult,
                    )
                    j3 = s_pool.tile([P, TOPK], u32, tag="j3")
                    nc.vector.tensor_scalar(
                        out=j3[:], in0=vu[:], scalar1=2047, scalar2=None,
                        op0=mybir.AluOpType.bitwise_and,
                    )
                    df3u = s_pool.tile([P, TOPK], u32, tag="dfu")
                    nc.vector.tensor_scalar(
                        out=df3u[:], in0=vu[:], scalar1=11, scalar2=None,
                        op0=mybir.AluOpType.logical_shift_right,
                    )
                    df3 = s_pool.tile([P, TOPK], f32, tag="df")
                    nc.vector.tensor_copy(out=df3[:], in_=df3u[:])
                    dw = s_pool.tile([P, TOPK], f32, tag="dw")
                    nc.scalar.activation(
                        dw[:],
                        df3[:],
                        mybir.ActivationFunctionType.Relu,
                        bias=halfcol[:],
                        scale=-INV_SCALE2,
                    )
                    dw2 = s_pool.tile([P, TOPK], f32, tag="dw2")
                    nc.scalar.activation(
                        dw2[:], dw[:], mybir.ActivationFunctionType.Square
                    )
                    # diag(dw2_k) matrices for the weighted PE transpose
                    dmats = s_pool.tile([P, TOPK, P], bf16, tag="dmats")
                    for k in range(TOPK):
                        nc.scalar.mul(dmats[:, k, :], identb[:], dw2[:, k : k + 1])
                    # 3 indirect gathers (the SWDGE lowering supports exactly
                    # one offset per 128-partition descriptor set)
                    g = g_pool.tile([P, TOPK, DF], bf16, tag="g")
                    for k in range(TOPK):
                        nc.gpsimd.indirect_dma_start(
                            out=g[:, k, :],
                            out_offset=None,
                            in_=rtab[:],
                            in_offset=IndirectOffsetOnAxis(ap=j3[:, k : k + 1], axis=0),
                        )
                    state[t] = (dmats, g)

                def mlp_group(g0, g1):
                    ntile = g1 - g0
                    accT0 = a_pool.tile([P, GRP * P], bf16, tag="accT0")
                    accT1 = a_pool.tile([P, GRP * P], bf16, tag="accT1")
                    for i, s in enumerate(range(g0, g1)):
                        dmats, g = state.pop(s)
                        # accT_m = sum_k (g_k[:, m-half])^T @ diag(dw2_k):
                        # weighted sum and transpose in one PE accumulation
                        for m, accTm in enumerate((accT0, accT1)):
                            pt = t_pool.tile([P, P], f32)
                            for k in range(TOPK):
                                nc.tensor.matmul(
                                    pt[:],
                                    lhsT=g[:, k, m * P : (m + 1) * P],
                                    rhs=dmats[:, k, :],
                                    start=(k == 0),
                                    stop=(k == TOPK - 1),
                                )
                            nc.scalar.copy(accTm[:, bass.ts(i, P)], pt[:])
                    csl = slice(g0 * P, g1 * P)
                    for m in range(2):
                        msl = bass.ts(m, P)
                        pf = f_pool.tile([P, GRP * P], f32)
                        nc.tensor.matmul(
                            pf[:, : ntile * P],
                            lhsT=w2k0[:, msl],
                            rhs=accT0[:, : ntile * P],
                            start=True,
                            stop=False,
                        )
                        nc.tensor.matmul(
                            pf[:, : ntile * P],
                            lhsT=w2k1[:, msl],
                            rhs=accT1[:, : ntile * P],
                            start=False,
                            stop=True,
                        )
                        oT = o_pool.tile([P, GRP * P], f32)
                        nc.scalar.activation(
                            oT[:, : ntile * P],
                            pf[:, : ntile * P],
                            mybir.ActivationFunctionType.Identity,
                            bias=b2s[:, m : m + 1],
                        )
                        nc.sync.dma_start(fusedT[msl, csl], oT[:, : ntile * P])

                SKEW = 2
                done = 0
                for t0 in range(0, NT + SKEW + 1, 2):
                    for t in (t0, t0 + 1):
                        if t < NT:
                            topk_and_gather(t)
                    t = min(t0 + 1, NT + SKEW)
                    # flush any complete group whose gathers are >= SKEW old
                    while done < NT and ((min(done + GRP, NT) - 1) + SKEW <= t):
                        g1 = min(done + GRP, NT)
                        mlp_group(done, g1)
                        done = g1
    nc.compile()
    return nc


def get_program():
    if "nc" not in _PROGRAM_CACHE:
        _PROGRAM_CACHE["nc"] = _build_program()
    return _PROGRAM_CACHE["nc"]


def _host_prep(batch_a, coords_a, batch_b, coords_b, feats_b, W1, b1, W2, b2):
    """Group by batch, build per-core input arrays. Returns (in_maps, meta)."""
    pa = np.argsort(batch_a, kind="stable")
    pb = np.argsort(batch_b, kind="stable")
    ca = np.bincount(batch_a, minlength=N_CORES)
    cb = np.bincount(batch_b, minlength=N_CORES)
    oa = np.concatenate([[0], np.cumsum(ca)])
    ob = np.concatenate([[0], np.cumsum(cb)])

    b2c3 = np.ascontiguousarray((3.0 * b2).astype(np.float32).reshape(2, P).T)
    import ml_dtypes
    bf = ml_dtypes.bfloat16
    w1c = np.ascontiguousarray(W1.astype(bf))
    w2c = np.ascontiguousarray(W2.astype(bf))

    in_maps = []
    meta = []
    for g in range(N_CORES):
        a_idx = pa[oa[g] : oa[g + 1]]
        b_idx = pb[ob[g] : ob[g + 1]]
        na, nb = len(a_idx), len(b_idx)
        if na > NPAD or nb > NPAD or (0 < nb < TOPK):
            return None, None  # shapes outside the compiled envelope -> fallback
        xa = (coords_a[a_idx] // 16).astype(np.float32)
        xb = (coords_b[b_idx] // 16).astype(np.float32)

        uaT = np.zeros((102, NPAD), dtype=np.float32)
        ua6 = uaT[0:6]
        ua6[1, :] = PACK
        ua6[5, :] = -1.0
        if na > 0:
            ua6[0, :na] = -PACK * np.square(xa).sum(1)
            ua6[2:5, :na] = (2.0 * PACK * xa).T
            # pad a-cols: copy of column 0 (harmless rows, outputs dropped)
            if na < NPAD:
                ua6[:, na:] = ua6[:, :1]

        vbT = np.zeros((102, NPAD), dtype=np.float32)
        vb6 = vbT[0:6]
        vb6[0, :] = 1.0
        vb6[1, :] = -BIG  # pad cols: huge distance, never selected
        vb6[5, :] = np.arange(NPAD, dtype=np.float32)
        if nb > 0:
            vb6[1, :nb] = -np.square(xb).sum(1)
            vb6[2:5, :nb] = xb.T
        for gofs in (32, 64, 96):
            uaT[gofs : gofs + 6] = ua6
            vbT[gofs : gofs + 6] = vb6

        fbT = np.zeros((DF, NPAD), dtype=bf)
        if nb > 0:
            fbT[:, :nb] = feats_b[b_idx].T.astype(bf)

        in_maps.append(
            {
                "uaT": uaT,
                "vbT": vbT,
                "fbT": fbT,
                "w1": w1c,
                "w2": w2c,
                "b2c3": b2c3,
            }
        )
        meta.append((a_idx, na, nb))
    return in_maps, meta


def _reference_numpy(batch_a, coords_a, feats_a, batch_b, coords_b, feats_b,
                     W1, b1, W2, b2):
    """Exact numpy fallback (mirrors reference.py) for out-of-envelope data."""
    xa = (coords_a // 16).astype(np.float32)
    xb = (coords_b // 16).astype(np.float32)
    d = (
        np.square(xa).sum(1)[:, None]
        + np.square(xb).sum(1)[None, :]
        - 2.0 * (xa @ xb.T)
    )
    d = np.clip(d, 0.0, None) / (FULL_SCALE**2)
    same = batch_a[:, None] == batch_b[None, :]
    d = np.where(same, d, np.inf)
    idx = np.argsort(d, axis=1, kind="stable")[:, :TOPK]
    dv = np.take_along_axis(d, idx, axis=1)
    dwt = RCLIP - np.clip(dv, 0.0, RCLIP)
    b_f = feats_b[idx] * dwt[..., None]
    h = np.maximum(b_f @ W1 + b1, 0.0) * dwt[..., None]
    fused = (h @ W2 + b2).sum(axis=1)
    return np.concatenate([feats_a, fused], axis=1).astype(np.float32)


def _ensure_ntff_hook():
    """Install the axon NTFF profile hook (missing antenv.axon_hooks shim)."""
    import sys
    import types

    if "antenv.axon_hooks" in sys.modules:
        return
    try:
        from trn_agent_boot.trn_boot import _ntff_profile_via_ctypes

        hook = _ntff_profile_via_ctypes("/opt/axon/libaxon_pjrt.so")
    except Exception:
        hook = None
    mod = types.ModuleType("antenv.axon_hooks")
    _state = {"hook": hook}
    mod.get_axon_ntff_profile_hook = lambda: _state["hook"]

    def _set(h):
        _state["hook"] = h

    mod.set_axon_ntff_profile_hook = _set
    sys.modules["antenv.axon_hooks"] = mod


def kernel(batch_a, coords_a, feats_a, batch_b, coords_b, feats_b, W1, b1, W2, b2):
    batch_a = np.asarray(batch_a)
    coords_a = np.asarray(coords_a)
    feats_a = np.asarray(feats_a, dtype=np.float32)
    batch_b = np.asarray(batch_b)
    coords_b = np.asarray(coords_b)
    feats_b = np.asarray(feats_b, dtype=np.float32)
    W1 = np.asarray(W1, dtype=np.float32)
    b1 = np.asarray(b1, dtype=np.float32)
    W2 = np.asarray(W2, dtype=np.float32)
    b2 = np.asarray(b2, dtype=np.float32)

    if np.any(b1 != 0.0):
        # device pipeline folds dw through relu; exact only for b1 == 0
        return _reference_numpy(
            batch_a, coords_a, feats_a, batch_b, coords_b, feats_b, W1, b1, W2, b2
        )

    in_maps, meta = _host_prep(
        batch_a, coords_a, batch_b, coords_b, feats_b, W1, b1, W2, b2
    )
    if in_maps is None:
        return _reference_numpy(
            batch_a, coords_a, feats_a, batch_b, coords_b, feats_b, W1, b1, W2, b2
        )

    nc = get_program()
    trace = bool(int(os.environ.get("KERNEL_TRACE", "0")))
    if trace:
        _ensure_ntff_hook()
    res = run_bass_kernel_spmd(
        nc, in_maps, core_ids=list(range(N_CORES)), trace=trace
    )
    kernel.last_results = res

    fused = np.zeros((len(batch_a), DF), dtype=np.float32)
    for g in range(N_CORES):
        a_idx, na, nb = meta[g]
        if na == 0:
            continue
        out_g = res.results[g]["fusedT"]  # [DF, NPAD]
        if nb == 0:
            # reference: dw=0 rows -> h=0 -> fused = 3*b2
            fused[a_idx] = 3.0 * b2
        else:
            fused[a_idx] = out_g[:, :na].T
    return np.concatenate([feats_a, fused], axis=1)
